# revision 3
# baseline (speedup 1.0000x reference)
"""Trainium2 Bass kernel for CheemsMambaMixer (Mamba-1 selective SSM mixer).

Shapes: B=1, L=2048, H=1024, DI=2048, DS=16, DTR=64, K=4.
Sharding: tensor-parallel over the d_inner channel dim (256 channels/core on
8 cores).  The only cross-core communication is a [96, 2048] fp32 AllReduce
of the x_proj partial products; the out_proj row-parallel partials are summed
on the host.

Everything device-side runs in fp16 storage with fp32 accumulation (PSUM,
scan state), which lands ~1e-3 relative error vs the fp32 reference.
"""
import sys

sys.path.insert(0, "/opt/trn_rl_repo")

import numpy as np

import concourse.bass as bass
import concourse.tile as tile
from concourse import mybir
from concourse.bass_utils import run_bass_kernel_spmd
from concourse.tile_rust import add_dep_helper
import bass_rust as _bass_rust

# ---------------------------------------------------------------- constants
N_CORES = 8
B, L, H = 1, 2048, 1024
DI, DS, DTR, K = 2048, 16, 64, 4
DIL = DI // N_CORES          # 256 channels per core
NDT = DIL // 128             # 2 d-tiles of 128 channels
LC = 512                     # time chunk
NCH = L // LC                # 4 chunks
NTILE = DIL * DS // 128      # 32 (d,n)-tiles per core, 8 d x 16 n each
TPG = NTILE // NDT           # 16 tiles per d-group

F16 = mybir.dt.float16
F32 = mybir.dt.float32

N_PROCS = 27


class _SplitDrainTileContext(tile.TileContext):
    """Tail drain split into single-wait drains: the CTRL_NO ISA struct holds
    one sync-wait, but a kernel using all 8 HWDGE queues plus a collective
    accumulates 9+ outstanding procs at the tail."""

    def _drain_and_barrier(self, tick_clock, wait_clock):
        full = tick_clock.global_clock
        ticks = [(i, full.peek_next(i) - 1) for i in range(N_PROCS)]
        ticks = [(i, v) for i, v in ticks if v > 0]
        for i, v in ticks:
            c = _bass_rust.VectorClock()
            c.require_at_least(i, v)
            drain_inst = self.nc.sync.drain(fusable=False)
            wait_clock.add_sem_waits(
                drain_inst.ins, _bass_rust.ScopedClock({None: c}))
        self.nc.all_engine_barrier()
        assert self.sems is not None
        popped = self.nc._tile_sem_poison_stack.pop()
        assert popped is self._sem_poison
        self.nc.clear_and_free_semaphores(list(self.sems.allocated().values()))
        self.nc.all_engine_barrier()


def _split_multi_waits(nc):
    """TPB ISA structs carry a single sync-wait slot; Tile sometimes attaches
    several.  Hoist all but the last wait of every instruction onto dedicated
    single-wait NoOps on the same engine, inserted just before it."""
    wid = 0
    for bb in nc.main_func.blocks:
        insts = list(bb.instructions)
        out = []
        changed = False
        for ins in insts:
            si = ins.sync_info
            if si is not None and si.on_wait and len(si.on_wait) > 1:
                waits = list(si.on_wait)
                for w in waits[:-1]:
                    nop = _bass_rust.InstNoOp(name=f"W-split-{wid}", ins=[],
                                              outs=[])
                    wid += 1
                    nop.engine = ins.engine
                    nop.sync_info = mybir.SyncInfo(on_wait=[w], on_update=[])
                    out.append(nop)
                ins.sync_info = mybir.SyncInfo(on_wait=[waits[-1]],
                                               on_update=list(si.on_update or []))
                changed = True
            out.append(ins)
        if changed:
            bb.instructions = out


# ---------------------------------------------------------------- builder
def _build(single_core=False, skip_scan=False):
    nc = bass.Bass("TRN2", target_bir_lowering=False, debug=False,
                   num_devices=N_CORES)
    Act = mybir.ActivationFunctionType
    Op = mybir.AluOpType

    def din(name, shape, dtype=F16):
        return nc.dram_tensor(name, shape, dtype, kind="ExternalInput").ap()

    hsT = din("hsT", [H, L])                       # hidden_states[0].T
    wxzT = din("wxzT", [H, 2 * DIL])               # in_proj rows (x|z).T slice
    owT = din("owT", [DIL, H])                     # out_proj.T slice
    xpwT = din("xpwT", [DIL, 96])                  # x_proj.T slice
    dtwT = din("dtwT", [DTR, DIL])                 # dt_proj.T slice
    selrep = din("selrep", [128, TPG, 128])        # SelRep[k, li, p]
    selmap = din("selmap", [128, TPG, 128])        # Selmap[p, li, m]
    selbc = din("selbc", [96, 2, 128])             # SelB / SelC
    acols = din("acols", [128, NTILE], F32)        # A[d,n] per (tile, partition)
    convw = din("convw", [128, NDT, K], F32)
    convb = din("convb", [128, NDT], F32)
    dtb = din("dtb", [128, NDT], F32)
    ddiag = din("ddiag", [128, NDT, 128])   # diag(D) per d-group
    out = nc.dram_tensor("out", [L, H], F32, kind="ExternalOutput").ap()

    with _SplitDrainTileContext(nc) as tc:
        import contextlib
        stack = contextlib.ExitStack()
        with stack:
            wpool = stack.enter_context(tc.tile_pool(name="wpool", bufs=1))
            state = stack.enter_context(tc.tile_pool(name="state", bufs=1))
            work = stack.enter_context(tc.tile_pool(name="work", bufs=3))
            psum = stack.enter_context(
                tc.tile_pool(name="psum", bufs=2, space="PSUM"))
            dram = stack.enter_context(
                tc.tile_pool(name="dram", bufs=1, space="DRAM"))

            # ---------------- load weights/constants
            wxzT_sb = wpool.tile([128, H // 128, 2 * DIL], F16)
            nc.sync.dma_start(wxzT_sb, wxzT.rearrange("(k p) m -> p k m", p=128))
            owT_sb = wpool.tile([128, NDT, H], F16)
            nc.sync.dma_start(owT_sb, owT.rearrange("(k p) h -> p k h", p=128))
            xpwT_sb = wpool.tile([128, NDT, 96], F16)
            nc.sync.dma_start(xpwT_sb, xpwT.rearrange("(k p) j -> p k j", p=128))
            dtwT_sb = wpool.tile([DTR, NDT, 128], F16)
            nc.sync.dma_start(dtwT_sb, dtwT.rearrange("k (m p) -> k m p", p=128))
            selrep_sb = wpool.tile([128, TPG, 128], F16)
            nc.sync.dma_start(selrep_sb, selrep)
            selmap_sb = wpool.tile([128, TPG, 128], F16)
            nc.sync.dma_start(selmap_sb, selmap)
            selbc_sb = wpool.tile([96, 2, 128], F16)
            nc.sync.dma_start(selbc_sb, selbc)
            acols_sb = wpool.tile([128, NTILE], F32)
            nc.sync.dma_start(acols_sb, acols)
            convw_sb = wpool.tile([128, NDT, K], F32)
            convw_dma = nc.sync.dma_start(convw_sb, convw)
            convb_sb = wpool.tile([128, NDT], F32)
            nc.sync.dma_start(convb_sb, convb)
            dtb_sb = wpool.tile([128, NDT], F32)
            nc.sync.dma_start(dtb_sb, dtb)
            ddiag_sb = wpool.tile([128, NDT, 128], F16)
            nc.sync.dma_start(ddiag_sb, ddiag)

            # Wait-slot fencing: TensorScalarPtr-class DVE ops (tensor_scalar,
            # scalar_tensor_tensor, tensor_tensor_scan) have very few sync-wait
            # slots in their ISA structs.  A tiny TensorTensor op (2 wait
            # slots) placed just before makes the in-order DVE observe the
            # producers' semaphores so the fragile op needs no new waits.
            fence_scratch = wpool.tile([128, 4], F32)

            def dve_observe(*insts):
                insts = [i for i in insts if i is not None]
                for j in range(0, len(insts), 1):
                    f = nc.vector.tensor_mul(
                        fence_scratch[:, 0:1], fence_scratch[:, 0:1],
                        fence_scratch[:, 0:1])
                    for d in insts[j:j + 1]:
                        add_dep_helper(f.ins, d.ins, sync=True,
                                       reason="dve wait fence")

            # The ACT engine loads ONE spline-table set; none contains
            # exp+silu+softplus together.  natural_log_exp_and_others has
            # {exp, ln, copy, identity}, so silu and softplus are synthesized:
            #   softplus(x) = ln(1 + e^x)
            #   silu(v) = v * sigma(v),  sigma(v) = exp(-ln(1 + e^-v))
            def silu_into(dst, v, pool):
                t1 = pool.tile([128, L], F16, tag="silu_t1", name="t1", bufs=1)
                nc.scalar.activation(t1, v, Act.Exp, scale=-1.0)
                t2 = pool.tile([128, L], F16, tag="silu_t2", name="t2", bufs=1)
                nc.scalar.activation(t2, t1, Act.Ln, bias=1.0)
                t3 = pool.tile([128, L], F16, tag="silu_t3", name="t3", bufs=1)
                nc.scalar.activation(t3, t2, Act.Exp, scale=-1.0)
                nc.vector.tensor_mul(dst, v, t3)

            # persistent state tensors
            xc = [state.tile([128, L], F16, name=f"xc{i}") for i in range(NDT)]
            zsb = [state.tile([128, L], F16, name=f"zsb{i}") for i in range(NDT)]
            dt = [state.tile([128, L], F16, name=f"dt{i}") for i in range(NDT)]
            dtx = [state.tile([128, L], F16, name=f"dtx{i}") for i in range(NDT)]
            ssm_sb = state.tile([96, L], F32, name="ssm_sb")
            ssmr16 = state.tile([96, L], F16, name="ssmr16")

            # ---------------- phase 1: in_proj + conv + silu
            with tc.tile_pool(name="inproj", bufs=1) as inproj:
                hsT_sb = inproj.tile([128, H // 128, L], F16)
                nc.sync.dma_start(hsT_sb, hsT.rearrange("(k p) t -> p k t", p=128))
                xpad = [inproj.tile([128, K - 1 + L], F16, name=f"xpad{i}")
                        for i in range(NDT)]
                xpad_evacs = [[] for _ in range(NDT)]
                for i in range(NDT):
                    nc.vector.memset(xpad[i][:, 0:K - 1], 0.0)

                for dm in range(2 * NDT):
                    for tch in range(NCH):
                        ps = psum.tile([128, LC], F32, tag="mm")
                        for k in range(H // 128):
                            nc.tensor.matmul(
                                ps,
                                lhsT=wxzT_sb[:, k, 128 * dm:128 * (dm + 1)],
                                rhs=hsT_sb[:, k, LC * tch:LC * (tch + 1)],
                                start=(k == 0), stop=(k == H // 128 - 1))
                        if dm < NDT:  # x branch -> conv input
                            ev = nc.scalar.copy(
                                xpad[dm][:, K - 1 + LC * tch:K - 1 + LC * (tch + 1)],
                                ps)
                            xpad_evacs[dm].append(ev)
                        else:         # z branch -> SBUF, silu later
                            nc.scalar.copy(
                                zsb[dm - NDT][:, LC * tch:LC * (tch + 1)], ps)

                # causal depthwise conv (K=4) + bias + silu
                # (z-branch silu comes AFTER: it is off the critical path)
                for i in range(NDT):
                    acc = inproj.tile([128, L], F16, tag="convacc", bufs=2, name="acc")
                    dve_observe(*xpad_evacs[i], convw_dma)
                    nc.vector.tensor_scalar(
                        acc, xpad[i][:, 0:L], convw_sb[:, i, 0:1], None,
                        op0=Op.mult)
                    for k in range(1, K):
                        nc.vector.scalar_tensor_tensor(
                            acc, xpad[i][:, k:k + L], convw_sb[:, i, k:k + 1],
                            acc, op0=Op.mult, op1=Op.add)
                    nc.vector.tensor_scalar_add(acc, acc, convb_sb[:, i:i + 1])
                    silu_into(xc[i], acc, inproj)

                for i in range(NDT):
                    silu_into(zsb[i], zsb[i], inproj)

            # scan-phase persistents allocated after inproj released its zone
            scanp = stack.enter_context(tc.tile_pool(name="scanp", bufs=1))
            brep = scanp.tile([128, L], F16, name="brep")
            crep = scanp.tile([128, L], F16, name="crep")
            ysb = [scanp.tile([128, L], F16, name=f"ysb{g}") for g in range(NDT)]
            yg = [scanp.tile([128, L], F16, name=f"yg{g}") for g in range(NDT)]

            # ---------------- phase 2: x_proj partial + AllReduce
            for tch in range(NCH):
                ps = psum.tile([128, LC], F32, tag="mm", name="ssm_ps")
                for ki in range(NDT):
                    nc.tensor.matmul(
                        ps[0:96, :], lhsT=xpwT_sb[:, ki, :],
                        rhs=xc[ki][:, LC * tch:LC * (tch + 1)],
                        start=(ki == 0), stop=(ki == NDT - 1))
                nc.scalar.copy(ssm_sb[:, LC * tch:LC * (tch + 1)], ps[0:96, :])

            ar_in = dram.tile([96, L], F32)
            ar_out = dram.tile([96, L], F32)
            nc.sync.dma_start(ar_in, ssm_sb)
            if single_core:
                nc.sync.dma_start(ar_out, ar_in)
            else:
                nc.gpsimd.collective_compute(
                    "AllReduce", Op.add,
                    replica_groups=[list(range(N_CORES))],
                    ins=[ar_in.opt()], outs=[ar_out.opt()])
            ssmr_sb = state.tile([96, L], F32, name="ssmr_sb")
            nc.sync.dma_start(ssmr_sb, ar_out)
            nc.scalar.copy(ssmr16, ssmr_sb)

            # ---------------- phase 3: dt = softplus(dt_proj @ dtr + b); dtx
            for mi in range(NDT):
                for tch in range(NCH):
                    ps = psum.tile([128, LC], F32, tag="mm", name="dt_ps")
                    nc.tensor.matmul(
                        ps, lhsT=dtwT_sb[:, mi, :],
                        rhs=ssmr16[0:DTR, LC * tch:LC * (tch + 1)],
                        start=True, stop=True)
                    # softplus(x+b) = ln(1 + e^(x+b)) via the exp/ln table set
                    spe = work.tile([128, LC], F32, tag="spe", bufs=2,
                                    name="spe")
                    nc.scalar.activation(spe, ps, Act.Exp,
                                         bias=dtb_sb[:, mi:mi + 1])
                    nc.scalar.activation(
                        dt[mi][:, LC * tch:LC * (tch + 1)], spe, Act.Ln,
                        bias=1.0)
            for i in range(NDT):
                nc.vector.tensor_mul(dtx[i], dt[i], xc[i])

            # ---------------- phase 4: B_rep / C_rep (shared across d-tiles)
            for tch in range(NCH):
                for j, dest in ((0, brep), (1, crep)):
                    ps = psum.tile([128, LC], F32, tag="mm", name="bc_ps")
                    nc.tensor.matmul(ps, lhsT=selbc_sb[:, j, :],
                                     rhs=ssmr16[:, LC * tch:LC * (tch + 1)],
                                     start=True, stop=True)
                    nc.scalar.copy(dest[:, LC * tch:LC * (tch + 1)], ps)

            # ---------------- phase 5: the scan
            # Full-L scans: per (d,n)-tile, build dA/dBx for all 2048 steps,
            # run one tensor_tensor_scan, multiply by C on GPSIMD, and reduce
            # n via selection matmuls accumulating dense y per chunk bank.
            for g in range(NDT):
                if skip_scan:
                    nc.scalar.copy(ysb[g], xc[g])
                    continue
                yac = [psum.tile([128, LC], F32, tag="yac", bufs=4,
                                 name=f"yac{c}") for c in range(NCH)]
                for li in range(TPG):
                    i = TPG * g + li
                    dA = work.tile([128, L], F16, tag="dA", bufs=3)
                    last_exp = None
                    for c in range(NCH):
                        tsl = slice(LC * c, LC * (c + 1))
                        drep = psum.tile([128, LC], F32, tag="mm", name="drep")
                        nc.tensor.matmul(drep, lhsT=selrep_sb[:, li, :],
                                         rhs=dt[g][:, tsl],
                                         start=True, stop=True)
                        last_exp = nc.scalar.activation(
                            dA[:, tsl], drep, Act.Exp,
                            scale=acols_sb[:, i:i + 1])
                    dBx = work.tile([128, L], F16, tag="dBx", bufs=3)
                    for half in range(2):
                        hsl = slice(1024 * half, 1024 * (half + 1))
                        dxp = psum.tile([128, 1024], F32, tag="dxrep", bufs=1,
                                        name="dxp")
                        for cc in range(2):
                            nc.tensor.matmul(
                                dxp[:, LC * cc:LC * (cc + 1)],
                                lhsT=selrep_sb[:, li, :],
                                rhs=dtx[g][:, 1024 * half + LC * cc:
                                           1024 * half + LC * (cc + 1)],
                                start=True, stop=True)
                        dbx_inst = nc.vector.tensor_mul(
                            dBx[:, hsl], dxp, brep[:, hsl])
                        if half == 0:
                            add_dep_helper(dbx_inst.ins, last_exp.ins,
                                           sync=True,
                                           reason="absorb ACT wait for scan")
                    hv = work.tile([128, L], F16, tag="hv", bufs=3)
                    nc.vector.tensor_tensor_scan(
                        hv, dA, dBx, 0.0, op0=Op.mult, op1=Op.add)
                    hc = work.tile([128, L], F16, tag="hc", bufs=3)
                    nc.vector.tensor_mul(hc, hv, crep)
                    for c in range(NCH):
                        tsl = slice(LC * c, LC * (c + 1))
                        nc.tensor.matmul(yac[c], lhsT=selmap_sb[:, li, :],
                                         rhs=hc[:, tsl],
                                         start=(li == 0), stop=False)
                for c in range(NCH):
                    tsl = slice(LC * c, LC * (c + 1))
                    # y += D * xc folded in as a diagonal matmul
                    nc.tensor.matmul(yac[c], lhsT=ddiag_sb[:, g, :],
                                     rhs=xc[g][:, tsl], start=False, stop=True)
                    nc.scalar.copy(ysb[g][:, tsl], yac[c])

            # ---------------- phase 6: gating + out_proj
            for g in range(NDT):
                nc.vector.tensor_mul(yg[g], ysb[g], zsb[g])

            for tb in range(L // 128):
                for hch in range(H // LC):
                    ops = psum.tile([128, LC], F32, tag="mm", name="out_ps")
                    for g in range(NDT):
                        nc.tensor.matmul(
                            ops, lhsT=yg[g][:, 128 * tb:128 * (tb + 1)],
                            rhs=owT_sb[:, g, LC * hch:LC * (hch + 1)],
                            start=(g == 0), stop=(g == NDT - 1))
                    osb = work.tile([128, LC], F32, tag="osb")
                    nc.scalar.copy(osb, ops)
                    nc.sync.dma_start(
                        out[128 * tb:128 * (tb + 1), LC * hch:LC * (hch + 1)],
                        osb)
    _split_multi_waits(nc)
    return nc


_NC_CACHE = None


def _get_nc():
    global _NC_CACHE
    if _NC_CACHE is None:
        _NC_CACHE = _build()
    return _NC_CACHE


# ---------------------------------------------------------------- host side
def _make_in_maps(hidden_states, in_proj_w, conv_w, conv_b, x_proj_w,
                  dt_proj_w, dt_proj_b, A_log, D, out_proj_w):
    hsT16 = np.ascontiguousarray(hidden_states[0].T, dtype=np.float16)

    # selection matrices (shared by all cores)
    p = np.arange(128)
    li = np.arange(TPG)
    k = np.arange(128)
    # SelRep[k, li, p] = 1 iff k == 8*li + p//16
    selrep = (k[:, None, None] == 8 * li[None, :, None] +
              (p // 16)[None, None, :]).astype(np.float16)
    # Selmap[p, li, m] = 1 iff m == 8*li + p//16
    selmap = (k[None, None, :] == 8 * li[None, :, None] +
              (p // 16)[:, None, None]).astype(np.float16)
    k96 = np.arange(96)
    selb = (k96[:, None] == 64 + (p % 16)[None, :])
    selc = (k96[:, None] == 80 + (p % 16)[None, :])
    selbc = np.stack([selb, selc], axis=1).astype(np.float16)

    A = -np.exp(np.asarray(A_log, np.float64))     # [DI, DS]

    in_maps = []
    for c in range(N_CORES):
        s = slice(DIL * c, DIL * (c + 1))
        wxz = np.concatenate(
            [in_proj_w[s], in_proj_w[DI + DIL * c:DI + DIL * (c + 1)]], axis=0)
        Ac = A[s]                                   # [256, 16]
        ti = np.arange(NTILE)
        acols = Ac[8 * ti[None, :] + (p // 16)[:, None], (p % 16)[:, None]]
        in_maps.append({
            "hsT": hsT16,
            "wxzT": np.ascontiguousarray(wxz.T, dtype=np.float16),
            "owT": np.ascontiguousarray(out_proj_w[:, s].T, dtype=np.float16),
            "xpwT": np.ascontiguousarray(x_proj_w[:, s].T, dtype=np.float16),
            "dtwT": np.ascontiguousarray(dt_proj_w[s].T, dtype=np.float16),
            "selrep": selrep, "selmap": selmap, "selbc": selbc,
            "acols": np.ascontiguousarray(acols, np.float32),
            "convw": np.ascontiguousarray(
                conv_w[s, 0, :].reshape(NDT, 128, K).transpose(1, 0, 2),
                np.float32),
            "convb": np.ascontiguousarray(
                conv_b[s].reshape(NDT, 128).T, np.float32),
            "dtb": np.ascontiguousarray(
                dt_proj_b[s].reshape(NDT, 128).T, np.float32),
            "ddiag": np.ascontiguousarray(
                np.einsum("gp,pm->pgm", D[s].reshape(NDT, 128),
                          np.eye(128)), np.float16),
        })
    return in_maps


def kernel(hidden_states, in_proj_w, conv_w, conv_b, x_proj_w,
           dt_proj_w, dt_proj_b, A_log, D, out_proj_w):
    args = [np.asarray(a, np.float32) for a in
            (hidden_states, in_proj_w, conv_w, conv_b, x_proj_w,
             dt_proj_w, dt_proj_b, A_log, D, out_proj_w)]
    in_maps = _make_in_maps(*args)
    nc = _get_nc()
    res = run_bass_kernel_spmd(nc, in_maps, core_ids=list(range(N_CORES)))
    out = np.zeros((L, H), np.float64)
    for r in res.results:
        out += r["out"].astype(np.float64)
    return out.astype(np.float32).reshape(B, L, H)



# revision 8
# speedup vs baseline: 1.0115x; 1.0115x over previous
"""Trainium2 Bass kernel for CheemsMambaMixer (Mamba-1 selective SSM mixer).

Shapes: B=1, L=2048, H=1024, DI=2048, DS=16, DTR=64, K=4.
Sharding: tensor-parallel over the d_inner channel dim (256 channels/core on
8 cores).  The only cross-core communication is a [96, 2048] fp16 AllReduce
of the x_proj partial products; the out_proj row-parallel partials are summed
on the host.

Device-side storage is fp16 with fp32 accumulation (PSUM).  Elementwise work
is split between the DVE (vector) and Pool (gpsimd) engines: the 32 scan
tiles alternate engines whole-tile, and the conv / dtx / gating muls split by
d-group, which roughly halves the previously DVE-bound scan phase.
"""
import sys

sys.path.insert(0, "/opt/trn_rl_repo")

import numpy as np

import concourse.bass as bass
import concourse.tile as tile
from concourse import mybir
from concourse.bass_utils import run_bass_kernel_spmd
from concourse.tile_rust import add_dep_helper
import bass_rust as _bass_rust

# ---------------------------------------------------------------- constants
N_CORES = 8
B, L, H = 1, 2048, 1024
DI, DS, DTR, K = 2048, 16, 64, 4
DIL = DI // N_CORES          # 256 channels per core
NDT = DIL // 128             # 2 d-tiles of 128 channels
LC = 512                     # time chunk
NCH = L // LC                # 4 chunks
NTILE = DIL * DS // 128      # 32 (d,n)-tiles per core, 8 d x 16 n each
TPG = NTILE // NDT           # 16 tiles per d-group

F16 = mybir.dt.float16
F32 = mybir.dt.float32

N_PROCS = 27


class _SplitDrainTileContext(tile.TileContext):
    """Tail drain split into single-wait drains: the CTRL_NO ISA struct holds
    one sync-wait, but a kernel using all 8 HWDGE queues plus a collective
    accumulates 9+ outstanding procs at the tail."""

    def _drain_and_barrier(self, tick_clock, wait_clock):
        full = tick_clock.global_clock
        ticks = [(i, full.peek_next(i) - 1) for i in range(N_PROCS)]
        ticks = [(i, v) for i, v in ticks if v > 0]
        for i, v in ticks:
            c = _bass_rust.VectorClock()
            c.require_at_least(i, v)
            drain_inst = self.nc.sync.drain(fusable=False)
            wait_clock.add_sem_waits(
                drain_inst.ins, _bass_rust.ScopedClock({None: c}))
        self.nc.all_engine_barrier()
        assert self.sems is not None
        popped = self.nc._tile_sem_poison_stack.pop()
        assert popped is self._sem_poison
        self.nc.clear_and_free_semaphores(list(self.sems.allocated().values()))
        self.nc.all_engine_barrier()


def _split_multi_waits(nc):
    """TPB ISA structs carry a single sync-wait slot; Tile sometimes attaches
    several.  Hoist all but the last wait of every instruction onto dedicated
    single-wait NoOps on the same engine, inserted just before it."""
    wid = 0
    for bb in nc.main_func.blocks:
        insts = list(bb.instructions)
        out = []
        changed = False
        for ins in insts:
            si = ins.sync_info
            if si is not None and si.on_wait and len(si.on_wait) > 1:
                waits = list(si.on_wait)
                for w in waits[:-1]:
                    nop = _bass_rust.InstNoOp(name=f"W-split-{wid}", ins=[],
                                              outs=[])
                    wid += 1
                    nop.engine = ins.engine
                    nop.sync_info = mybir.SyncInfo(on_wait=[w], on_update=[])
                    out.append(nop)
                ins.sync_info = mybir.SyncInfo(on_wait=[waits[-1]],
                                               on_update=list(si.on_update or []))
                changed = True
            out.append(ins)
        if changed:
            bb.instructions = out


# ---------------------------------------------------------------- builder
def _build(single_core=False, skip_scan=False):
    nc = bass.Bass("TRN2", target_bir_lowering=False, debug=False,
                   num_devices=N_CORES)
    Act = mybir.ActivationFunctionType
    Op = mybir.AluOpType

    def din(name, shape, dtype=F16):
        return nc.dram_tensor(name, shape, dtype, kind="ExternalInput").ap()

    hsT = din("hsT", [H, L])                       # hidden_states[0].T
    wxzT = din("wxzT", [H, 2 * DIL])               # in_proj rows (x|z).T slice
    owT = din("owT", [DIL, H])                     # out_proj.T slice
    xpwT = din("xpwT", [DIL, 96])                  # x_proj.T slice
    dtwT = din("dtwT", [DTR, DIL])                 # dt_proj.T slice
    selrep = din("selrep", [128, TPG, 128])        # SelRep[k, li, p]
    selmap = din("selmap", [128, TPG, 128])        # Selmap[p, li, m]
    selbc = din("selbc", [96, 2, 128])             # SelB / SelC
    acols = din("acols", [128, NTILE], F32)        # A[d,n] per (tile, partition)
    convw = din("convw", [128, NDT, K], F32)
    convb = din("convb", [128, NDT], F32)
    dtb = din("dtb", [128, NDT], F32)
    ddiag = din("ddiag", [128, NDT, 128])   # diag(D) per d-group
    out = nc.dram_tensor("out", [L, H], F16, kind="ExternalOutput").ap()

    with _SplitDrainTileContext(nc) as tc:
        import contextlib
        stack = contextlib.ExitStack()
        with stack:
            wpool = stack.enter_context(tc.tile_pool(name="wpool", bufs=1))
            state = stack.enter_context(tc.tile_pool(name="state", bufs=1))
            work = stack.enter_context(tc.tile_pool(name="work", bufs=3))
            psum = stack.enter_context(
                tc.tile_pool(name="psum", bufs=3, space="PSUM"))
            dram = stack.enter_context(
                tc.tile_pool(name="dram", bufs=1, space="DRAM"))

            # ---------------- load weights/constants
            wxzT_sb = wpool.tile([128, H // 128, 2 * DIL], F16)
            nc.sync.dma_start(wxzT_sb, wxzT.rearrange("(k p) m -> p k m", p=128))
            owT_sb = wpool.tile([128, NDT, H], F16)
            nc.sync.dma_start(owT_sb, owT.rearrange("(k p) h -> p k h", p=128))
            xpwT_sb = wpool.tile([128, NDT, 96], F16)
            nc.sync.dma_start(xpwT_sb, xpwT.rearrange("(k p) j -> p k j", p=128))
            dtwT_sb = wpool.tile([DTR, NDT, 128], F16)
            nc.sync.dma_start(dtwT_sb, dtwT.rearrange("k (m p) -> k m p", p=128))
            selrep_sb = wpool.tile([128, TPG, 128], F16)
            nc.sync.dma_start(selrep_sb, selrep)
            selmap_sb = wpool.tile([128, TPG, 128], F16)
            nc.sync.dma_start(selmap_sb, selmap)
            selbc_sb = wpool.tile([96, 2, 128], F16)
            nc.sync.dma_start(selbc_sb, selbc)
            acols_sb = wpool.tile([128, NTILE], F32)
            nc.sync.dma_start(acols_sb, acols)
            convw_sb = wpool.tile([128, NDT, K], F32)
            convw_dma = nc.sync.dma_start(convw_sb, convw)
            convb_sb = wpool.tile([128, NDT], F32)
            nc.sync.dma_start(convb_sb, convb)
            dtb_sb = wpool.tile([128, NDT], F32)
            nc.sync.dma_start(dtb_sb, dtb)
            ddiag_sb = wpool.tile([128, NDT, 128], F16)
            nc.sync.dma_start(ddiag_sb, ddiag)

            # Wait-slot fencing: TensorScalarPtr-class ops (tensor_scalar,
            # scalar_tensor_tensor, tensor_tensor_scan) have very few sync-wait
            # slots in their ISA structs.  A tiny TensorTensor op (2 wait
            # slots) placed just before makes the in-order engine observe the
            # producers' semaphores so the fragile op needs no new waits.
            fence_scratch = wpool.tile([128, 4], F32)

            def observe(eng, col, *insts):
                insts = [i for i in insts if i is not None]
                for j in range(len(insts)):
                    f = eng.tensor_mul(
                        fence_scratch[:, col:col + 1],
                        fence_scratch[:, col:col + 1],
                        fence_scratch[:, col:col + 1])
                    add_dep_helper(f.ins, insts[j].ins, sync=True,
                                   reason="wait fence")

            # persistent state tensors
            xc = [state.tile([128, L], F16, name=f"xc{i}") for i in range(NDT)]
            zsb = [state.tile([128, L], F16, name=f"zsb{i}") for i in range(NDT)]
            dt = [state.tile([128, L], F16, name=f"dt{i}") for i in range(NDT)]
            dtx = [state.tile([128, L], F16, name=f"dtx{i}") for i in range(NDT)]
            ssm16 = state.tile([96, L], F16, name="ssm16")
            ssmr16 = state.tile([96, L], F16, name="ssmr16")

            # ---------------- phase 1: in_proj + conv + silu
            with tc.tile_pool(name="inproj", bufs=1) as inproj:
                hsT_sb = inproj.tile([128, H // 128, L], F16)
                nc.sync.dma_start(hsT_sb, hsT.rearrange("(k p) t -> p k t", p=128))
                xpad = [inproj.tile([128, K - 1 + L], F16, name=f"xpad{i}")
                        for i in range(NDT)]
                xpad_evacs = [[] for _ in range(NDT)]
                for i in range(NDT):
                    nc.vector.memset(xpad[i][:, 0:K - 1], 0.0)

                for dm in range(2 * NDT):
                    for tch in range(NCH):
                        ps = psum.tile([128, LC], F32, tag="mm")
                        for k in range(H // 128):
                            nc.tensor.matmul(
                                ps,
                                lhsT=wxzT_sb[:, k, 128 * dm:128 * (dm + 1)],
                                rhs=hsT_sb[:, k, LC * tch:LC * (tch + 1)],
                                start=(k == 0), stop=(k == H // 128 - 1))
                        if dm < NDT:  # x branch -> conv input
                            ev = nc.scalar.copy(
                                xpad[dm][:, K - 1 + LC * tch:K - 1 + LC * (tch + 1)],
                                ps)
                            xpad_evacs[dm].append(ev)
                        else:         # z branch -> silu folded into the evac
                            nc.scalar.activation(
                                zsb[dm - NDT][:, LC * tch:LC * (tch + 1)], ps,
                                Act.Silu)

                # causal depthwise conv (K=4) + bias; silu on the ACT engine.
                # TensorScalarPtr-class ops only exist on DVE, so the conv
                # muls stay there (DVE is idle during the in_proj phase).
                for i in range(NDT):
                    eng = nc.vector
                    acc = inproj.tile([128, L], F16, tag="convacc", bufs=2,
                                      name="acc")
                    observe(eng, i, *xpad_evacs[i], convw_dma)
                    eng.tensor_scalar(
                        acc, xpad[i][:, 0:L], convw_sb[:, i, 0:1], None,
                        op0=Op.mult)
                    for k in range(1, K):
                        eng.scalar_tensor_tensor(
                            acc, xpad[i][:, k:k + L], convw_sb[:, i, k:k + 1],
                            acc, op0=Op.mult, op1=Op.add)
                    eng.tensor_scalar_add(acc, acc, convb_sb[:, i:i + 1])
                    nc.scalar.activation(xc[i], acc, Act.Silu)

            # scan-phase persistents allocated after inproj released its zone
            scanp = stack.enter_context(tc.tile_pool(name="scanp", bufs=1))
            brep = scanp.tile([128, L], F16, name="brep")
            crep = scanp.tile([128, L], F16, name="crep")
            ysb = [scanp.tile([128, L], F16, name=f"ysb{g}") for g in range(NDT)]
            yg = [scanp.tile([128, L], F16, name=f"yg{g}") for g in range(NDT)]

            # ---------------- phase 2: x_proj partial + AllReduce (fp16)
            # The AllReduce is split in two L-halves so dt/B/C work on half 0
            # overlaps the reduction of half 1.
            LH = L // 2
            ar_in = [dram.tile([96, LH], F16, name=f"ar_in{h}")
                     for h in range(2)]
            ar_out = [dram.tile([96, LH], F16, name=f"ar_out{h}")
                      for h in range(2)]
            for half in range(2):
                for tch in (2 * half, 2 * half + 1):
                    ps = psum.tile([128, LC], F32, tag="mm", name="ssm_ps")
                    for ki in range(NDT):
                        nc.tensor.matmul(
                            ps[0:96, :], lhsT=xpwT_sb[:, ki, :],
                            rhs=xc[ki][:, LC * tch:LC * (tch + 1)],
                            start=(ki == 0), stop=(ki == NDT - 1))
                    nc.scalar.copy(ssm16[:, LC * tch:LC * (tch + 1)],
                                   ps[0:96, :])
                nc.sync.dma_start(ar_in[half],
                                  ssm16[:, LH * half:LH * (half + 1)])
                if single_core:
                    nc.sync.dma_start(ar_out[half], ar_in[half])
                else:
                    nc.gpsimd.collective_compute(
                        "AllReduce", Op.add,
                        replica_groups=[list(range(N_CORES))],
                        ins=[ar_in[half].opt()], outs=[ar_out[half].opt()])
                nc.sync.dma_start(ssmr16[:, LH * half:LH * (half + 1)],
                                  ar_out[half])

            # ---------------- phase 3+4 per half: dt softplus, dtx, B/C rep
            for half in range(2):
                hsl = slice(LH * half, LH * (half + 1))
                for tch in (2 * half, 2 * half + 1):
                    tsl = slice(LC * tch, LC * (tch + 1))
                    for mi in range(NDT):
                        ps = psum.tile([128, LC], F32, tag="mm", name="dt_ps")
                        nc.tensor.matmul(
                            ps, lhsT=dtwT_sb[:, mi, :],
                            rhs=ssmr16[0:DTR, tsl],
                            start=True, stop=True)
                        # softplus(x+b) = ln(1 + e^(x+b)); no native softplus
                        # set in this compiler's act tables
                        spe = work.tile([128, LC], F32, tag="spe", bufs=2,
                                        name="spe")
                        nc.scalar.activation(spe, ps, Act.Exp,
                                             bias=dtb_sb[:, mi:mi + 1])
                        nc.scalar.activation(dt[mi][:, tsl], spe, Act.Ln,
                                             bias=1.0)
                    for j, dest in ((0, brep), (1, crep)):
                        ps = psum.tile([128, LC], F32, tag="mm", name="bc_ps")
                        nc.tensor.matmul(ps, lhsT=selbc_sb[:, j, :],
                                         rhs=ssmr16[:, tsl],
                                         start=True, stop=True)
                        nc.scalar.copy(dest[:, tsl], ps)
                for i in range(NDT):
                    eng = nc.vector if i == 0 else nc.gpsimd
                    eng.tensor_mul(dtx[i][:, hsl], dt[i][:, hsl],
                                   xc[i][:, hsl])

            # ---------------- phase 5: the scan
            # Full-L scans per (d,n)-tile.  Engine constraints: scans and
            # PSUM reads only exist on DVE; Pool only does plain SBUF
            # TensorTensor.  Split: every scan on DVE; for half the tiles the
            # dBx mul reads PSUM on DVE, for the other half ACT evacuates the
            # replicated dtx to SBUF and Pool does the mul; every hv*C mul on
            # Pool.  The C-mul + y-reduction of tile i-2 are issued after
            # tile i's expansion so the in-order PE queue never stalls.
            def build_tile(g, li):
                i = TPG * g + li
                act_evac = (i % 2 == 1)
                dA = work.tile([128, L], F16, tag="dA", bufs=3)
                last_exp = None
                for c in range(NCH):
                    tsl = slice(LC * c, LC * (c + 1))
                    drep = psum.tile([128, LC], F32, tag="mm", name="drep")
                    nc.tensor.matmul(drep, lhsT=selrep_sb[:, li, :],
                                     rhs=dt[g][:, tsl],
                                     start=True, stop=True)
                    last_exp = nc.scalar.activation(
                        dA[:, tsl], drep, Act.Exp,
                        scale=acols_sb[:, i:i + 1])
                dBx = work.tile([128, L], F16, tag="dBx", bufs=3)
                last_dbx = None
                if act_evac:
                    dxs = work.tile([128, L], F16, tag="dxs", bufs=2)
                for c in range(NCH):
                    tsl = slice(LC * c, LC * (c + 1))
                    dxp = psum.tile([128, LC], F32, tag="mm", name="dxp")
                    nc.tensor.matmul(dxp, lhsT=selrep_sb[:, li, :],
                                     rhs=dtx[g][:, tsl],
                                     start=True, stop=True)
                    if act_evac:
                        nc.scalar.copy(dxs[:, tsl], dxp)
                        last_dbx = nc.gpsimd.tensor_mul(
                            dBx[:, tsl], dxs[:, tsl], brep[:, tsl])
                    else:
                        last_dbx = nc.vector.tensor_mul(dBx[:, tsl], dxp,
                                                        brep[:, tsl])
                observe(nc.vector, 2, last_exp, last_dbx)
                hv = work.tile([128, L], F16, tag="hv", bufs=3)
                nc.vector.tensor_tensor_scan(
                    hv, dA, dBx, 0.0, op0=Op.mult, op1=Op.add)
                return hv

            def reduce_tile(g, li, hv, yac):
                hc = work.tile([128, L], F16, tag="hc", bufs=3)
                nc.gpsimd.tensor_mul(hc, hv, crep)
                for c in range(NCH):
                    tsl = slice(LC * c, LC * (c + 1))
                    nc.tensor.matmul(yac[c], lhsT=selmap_sb[:, li, :],
                                     rhs=hc[:, tsl],
                                     start=(li == 0), stop=False)

            for g in range(NDT):
                if skip_scan:
                    nc.scalar.copy(ysb[g], xc[g])
                    nc.vector.tensor_mul(yg[g], ysb[g], zsb[g])
                    continue
                yac = [psum.tile([128, LC], F32, tag="yac", bufs=4,
                                 name=f"yac{c}") for c in range(NCH)]
                pending = []
                for li in range(TPG):
                    pending.append((li, build_tile(g, li)))
                    if len(pending) > 2:
                        pli, phv = pending.pop(0)
                        reduce_tile(g, pli, phv, yac)
                for pli, phv in pending:
                    reduce_tile(g, pli, phv, yac)
                for c in range(NCH):
                    tsl = slice(LC * c, LC * (c + 1))
                    # y += D * xc folded in as a diagonal matmul
                    nc.tensor.matmul(yac[c], lhsT=ddiag_sb[:, g, :],
                                     rhs=xc[g][:, tsl], start=False, stop=True)
                    nc.scalar.copy(ysb[g][:, tsl], yac[c])
                # gating for this group (overlaps the other group's scan)
                geng = nc.vector if g == 0 else nc.gpsimd
                geng.tensor_mul(yg[g], ysb[g], zsb[g])

            # ---------------- phase 6: out_proj
            for tb in range(L // 128):
                for hch in range(H // LC):
                    ops = psum.tile([128, LC], F32, tag="mm", name="out_ps")
                    for g in range(NDT):
                        nc.tensor.matmul(
                            ops, lhsT=yg[g][:, 128 * tb:128 * (tb + 1)],
                            rhs=owT_sb[:, g, LC * hch:LC * (hch + 1)],
                            start=(g == 0), stop=(g == NDT - 1))
                    osb = work.tile([128, LC], F16, tag="osb")
                    nc.scalar.copy(osb, ops)
                    nc.sync.dma_start(
                        out[128 * tb:128 * (tb + 1), LC * hch:LC * (hch + 1)],
                        osb)
    _split_multi_waits(nc)
    return nc


_NC_CACHE = None


def _get_nc():
    global _NC_CACHE
    if _NC_CACHE is None:
        _NC_CACHE = _build()
    return _NC_CACHE


# ---------------------------------------------------------------- host side
def _make_in_maps(hidden_states, in_proj_w, conv_w, conv_b, x_proj_w,
                  dt_proj_w, dt_proj_b, A_log, D, out_proj_w):
    hsT16 = np.ascontiguousarray(hidden_states[0].T, dtype=np.float16)

    # selection matrices (shared by all cores)
    p = np.arange(128)
    li = np.arange(TPG)
    k = np.arange(128)
    # SelRep[k, li, p] = 1 iff k == 8*li + p//16
    selrep = (k[:, None, None] == 8 * li[None, :, None] +
              (p // 16)[None, None, :]).astype(np.float16)
    # Selmap[p, li, m] = 1 iff m == 8*li + p//16
    selmap = (k[None, None, :] == 8 * li[None, :, None] +
              (p // 16)[:, None, None]).astype(np.float16)
    k96 = np.arange(96)
    selb = (k96[:, None] == 64 + (p % 16)[None, :])
    selc = (k96[:, None] == 80 + (p % 16)[None, :])
    selbc = np.stack([selb, selc], axis=1).astype(np.float16)

    A = -np.exp(np.asarray(A_log, np.float64))     # [DI, DS]

    in_maps = []
    for c in range(N_CORES):
        s = slice(DIL * c, DIL * (c + 1))
        wxz = np.concatenate(
            [in_proj_w[s], in_proj_w[DI + DIL * c:DI + DIL * (c + 1)]], axis=0)
        Ac = A[s]                                   # [256, 16]
        ti = np.arange(NTILE)
        acols = Ac[8 * ti[None, :] + (p // 16)[:, None], (p % 16)[:, None]]
        in_maps.append({
            "hsT": hsT16,
            "wxzT": np.ascontiguousarray(wxz.T, dtype=np.float16),
            "owT": np.ascontiguousarray(out_proj_w[:, s].T, dtype=np.float16),
            "xpwT": np.ascontiguousarray(x_proj_w[:, s].T, dtype=np.float16),
            "dtwT": np.ascontiguousarray(dt_proj_w[s].T, dtype=np.float16),
            "selrep": selrep, "selmap": selmap, "selbc": selbc,
            "acols": np.ascontiguousarray(acols, np.float32),
            "convw": np.ascontiguousarray(
                conv_w[s, 0, :].reshape(NDT, 128, K).transpose(1, 0, 2),
                np.float32),
            "convb": np.ascontiguousarray(
                conv_b[s].reshape(NDT, 128).T, np.float32),
            "dtb": np.ascontiguousarray(
                dt_proj_b[s].reshape(NDT, 128).T, np.float32),
            "ddiag": np.ascontiguousarray(
                np.einsum("gp,pm->pgm", D[s].reshape(NDT, 128),
                          np.eye(128)), np.float16),
        })
    return in_maps


def kernel(hidden_states, in_proj_w, conv_w, conv_b, x_proj_w,
           dt_proj_w, dt_proj_b, A_log, D, out_proj_w):
    args = [np.asarray(a, np.float32) for a in
            (hidden_states, in_proj_w, conv_w, conv_b, x_proj_w,
             dt_proj_w, dt_proj_b, A_log, D, out_proj_w)]
    in_maps = _make_in_maps(*args)
    nc = _get_nc()
    res = run_bass_kernel_spmd(nc, in_maps, core_ids=list(range(N_CORES)))
    out = np.zeros((L, H), np.float64)
    for r in res.results:
        out += r["out"].astype(np.float64)
    return out.astype(np.float32).reshape(B, L, H)


# revision 14
# speedup vs baseline: 5.5438x; 5.4807x over previous
"""Trainium2 Bass kernel for CheemsMambaMixer (Mamba-1 selective SSM mixer).

Shapes: B=1, L=2048, H=1024, DI=2048, DS=16, DTR=64, K=4.
Sharding: tensor-parallel over the d_inner channel dim (256 channels/core on
8 cores).  The only cross-core communication is a [96, 2048] fp16 AllReduce
of the x_proj partial products; the out_proj row-parallel partials are summed
on the host.

Device-side storage is fp16 with fp32 accumulation (PSUM).  Elementwise work
is split between the DVE (vector) and Pool (gpsimd) engines: the 32 scan
tiles alternate engines whole-tile, and the conv / dtx / gating muls split by
d-group, which roughly halves the previously DVE-bound scan phase.
"""
import sys

sys.path.insert(0, "/opt/trn_rl_repo")

import numpy as np

import concourse.bass as bass
import concourse.tile as tile
from concourse import mybir
from concourse.bass_utils import run_bass_kernel_spmd
from concourse.tile_rust import add_dep_helper
import bass_rust as _bass_rust

# ---------------------------------------------------------------- constants
N_CORES = 8
B, L, H = 1, 2048, 1024
DI, DS, DTR, K = 2048, 16, 64, 4
DIL = DI // N_CORES          # 256 channels per core
NDT = DIL // 128             # 2 d-tiles of 128 channels
LC = 512                     # time chunk
NCH = L // LC                # 4 chunks
NTILE = DIL * DS // 128      # 32 (d,n)-tiles per core, 8 d x 16 n each
TPG = NTILE // NDT           # 16 tiles per d-group

F16 = mybir.dt.float16
F32 = mybir.dt.float32

# packed input blob sizes (elements); order defined in _build/_make_in_maps
BLOB16_SIZE = (H * L + H * 2 * DIL + DIL * H + DIL * 96 + DTR * DIL
               + 128 * TPG * 128 * 2 + 96 * 2 * 128 + 128 * NDT * 128)
BLOB32_SIZE = 128 * NTILE + 128 * NDT * K + 128 * NDT * 2

N_PROCS = 27


class _SplitDrainTileContext(tile.TileContext):
    """Tail drain split into single-wait drains: the CTRL_NO ISA struct holds
    one sync-wait, but a kernel using all 8 HWDGE queues plus a collective
    accumulates 9+ outstanding procs at the tail."""

    def _drain_and_barrier(self, tick_clock, wait_clock):
        full = tick_clock.global_clock
        ticks = [(i, full.peek_next(i) - 1) for i in range(N_PROCS)]
        ticks = [(i, v) for i, v in ticks if v > 0]
        for i, v in ticks:
            c = _bass_rust.VectorClock()
            c.require_at_least(i, v)
            drain_inst = self.nc.sync.drain(fusable=False)
            wait_clock.add_sem_waits(
                drain_inst.ins, _bass_rust.ScopedClock({None: c}))
        self.nc.all_engine_barrier()
        assert self.sems is not None
        popped = self.nc._tile_sem_poison_stack.pop()
        assert popped is self._sem_poison
        self.nc.clear_and_free_semaphores(list(self.sems.allocated().values()))
        self.nc.all_engine_barrier()


def _split_multi_waits(nc):
    """TPB ISA structs carry a single sync-wait slot; Tile sometimes attaches
    several.  Hoist all but the last wait of every instruction onto dedicated
    single-wait NoOps on the same engine, inserted just before it."""
    wid = 0
    for bb in nc.main_func.blocks:
        insts = list(bb.instructions)
        out = []
        changed = False
        for ins in insts:
            si = ins.sync_info
            if si is not None and si.on_wait and len(si.on_wait) > 1:
                waits = list(si.on_wait)
                for w in waits[:-1]:
                    nop = _bass_rust.InstNoOp(name=f"W-split-{wid}", ins=[],
                                              outs=[])
                    wid += 1
                    nop.engine = ins.engine
                    nop.sync_info = mybir.SyncInfo(on_wait=[w], on_update=[])
                    out.append(nop)
                ins.sync_info = mybir.SyncInfo(on_wait=[waits[-1]],
                                               on_update=list(si.on_update or []))
                changed = True
            out.append(ins)
        if changed:
            bb.instructions = out


# ---------------------------------------------------------------- builder
def _build(single_core=False, skip_scan=False):
    nc = bass.Bass("TRN2", target_bir_lowering=False, debug=False,
                   num_devices=N_CORES)
    Act = mybir.ActivationFunctionType
    Op = mybir.AluOpType

    # All inputs are packed into two 1-D blobs (f16 + f32): per-executable
    # argument count dominates the per-call runtime dispatch cost, so 13
    # tensors -> 2 saves ~1 ms/call of host-side overhead.
    blob16 = nc.dram_tensor("blob16", [BLOB16_SIZE], F16,
                            kind="ExternalInput").ap()
    blob32 = nc.dram_tensor("blob32", [BLOB32_SIZE], F32,
                            kind="ExternalInput").ap()

    off16 = [0]
    off32 = [0]

    def take(blob, off, shape, pattern=None, **axes):
        n = int(np.prod(shape))
        sl = blob[off[0]:off[0] + n]
        off[0] += n
        return sl.rearrange(pattern, **axes)

    # order must match _make_in_maps
    hsT = take(blob16, off16, [H // 128, 128, L],
               "(k p t) -> p k t", p=128, t=L)          # hidden_states.T
    wxzT = take(blob16, off16, [H, 2 * DIL],
                "(k p m) -> p k m", p=128, m=2 * DIL)   # in_proj (x|z).T
    owT = take(blob16, off16, [DIL, H],
               "(k p h) -> p k h", p=128, h=H)          # out_proj.T slice
    xpwT = take(blob16, off16, [DIL, 96],
                "(k p j) -> p k j", p=128, j=96)        # x_proj.T slice
    dtwT = take(blob16, off16, [DTR, DIL],
                "(k m p) -> k m p", m=NDT, p=128)       # dt_proj.T slice
    selrep = take(blob16, off16, [128, TPG, 128],
                  "(a b c) -> a b c", b=TPG, c=128)     # SelRep[k, li, p]
    selmap = take(blob16, off16, [128, TPG, 128],
                  "(a b c) -> a b c", b=TPG, c=128)     # Selmap[p, li, m]
    selbc = take(blob16, off16, [96, 2, 128],
                 "(a b c) -> a b c", b=2, c=128)        # SelB / SelC
    ddiag = take(blob16, off16, [128, NDT, 128],
                 "(a b c) -> a b c", b=NDT, c=128)      # diag(D) per d-group
    acols = take(blob32, off32, [128, NTILE],
                 "(a b) -> a b", b=NTILE)               # A[d,n] per tile col
    convw = take(blob32, off32, [128, NDT, K],
                 "(a b c) -> a b c", b=NDT, c=K)
    convb = take(blob32, off32, [128, NDT], "(a b) -> a b", b=NDT)
    dtb = take(blob32, off32, [128, NDT], "(a b) -> a b", b=NDT)
    assert off16[0] == BLOB16_SIZE and off32[0] == BLOB32_SIZE
    out = nc.dram_tensor("out", [L, H], F16, kind="ExternalOutput").ap()

    with _SplitDrainTileContext(nc) as tc:
        import contextlib
        stack = contextlib.ExitStack()
        with stack:
            wpool = stack.enter_context(tc.tile_pool(name="wpool", bufs=1))
            state = stack.enter_context(tc.tile_pool(name="state", bufs=1))
            work = stack.enter_context(tc.tile_pool(name="work", bufs=3))
            psum = stack.enter_context(
                tc.tile_pool(name="psum", bufs=3, space="PSUM"))
            dram = stack.enter_context(
                tc.tile_pool(name="dram", bufs=1, space="DRAM"))

            # ---------------- load weights/constants
            wxzT_sb = wpool.tile([128, H // 128, 2 * DIL], F16)
            nc.sync.dma_start(wxzT_sb, wxzT)
            owT_sb = wpool.tile([128, NDT, H], F16)
            nc.sync.dma_start(owT_sb, owT)
            xpwT_sb = wpool.tile([128, NDT, 96], F16)
            nc.sync.dma_start(xpwT_sb, xpwT)
            dtwT_sb = wpool.tile([DTR, NDT, 128], F16)
            nc.sync.dma_start(dtwT_sb, dtwT)
            selrep_sb = wpool.tile([128, TPG, 128], F16)
            nc.sync.dma_start(selrep_sb, selrep)
            selmap_sb = wpool.tile([128, TPG, 128], F16)
            nc.sync.dma_start(selmap_sb, selmap)
            selbc_sb = wpool.tile([96, 2, 128], F16)
            nc.sync.dma_start(selbc_sb, selbc)
            acols_sb = wpool.tile([128, NTILE], F32)
            nc.sync.dma_start(acols_sb, acols)
            convw_sb = wpool.tile([128, NDT, K], F32)
            convw_dma = nc.sync.dma_start(convw_sb, convw)
            convb_sb = wpool.tile([128, NDT], F32)
            nc.sync.dma_start(convb_sb, convb)
            dtb_sb = wpool.tile([128, NDT], F32)
            nc.sync.dma_start(dtb_sb, dtb)
            ddiag_sb = wpool.tile([128, NDT, 128], F16)
            nc.sync.dma_start(ddiag_sb, ddiag)

            # Wait-slot fencing: TensorScalarPtr-class ops (tensor_scalar,
            # scalar_tensor_tensor, tensor_tensor_scan) have very few sync-wait
            # slots in their ISA structs.  A tiny TensorTensor op (2 wait
            # slots) placed just before makes the in-order engine observe the
            # producers' semaphores so the fragile op needs no new waits.
            fence_scratch = wpool.tile([128, 4], F32)

            def observe(eng, col, *insts):
                insts = [i for i in insts if i is not None]
                for j in range(len(insts)):
                    f = eng.tensor_mul(
                        fence_scratch[:, col:col + 1],
                        fence_scratch[:, col:col + 1],
                        fence_scratch[:, col:col + 1])
                    add_dep_helper(f.ins, insts[j].ins, sync=True,
                                   reason="wait fence")

            # persistent state tensors
            xc = [state.tile([128, L], F16, name=f"xc{i}") for i in range(NDT)]
            zsb = [state.tile([128, L], F16, name=f"zsb{i}") for i in range(NDT)]
            dt = [state.tile([128, L], F16, name=f"dt{i}") for i in range(NDT)]
            dtx = [state.tile([128, L], F16, name=f"dtx{i}") for i in range(NDT)]
            ssm16 = state.tile([96, L], F16, name="ssm16")
            ssmr16 = state.tile([96, L], F16, name="ssmr16")

            # ---------------- phase 1: in_proj + conv + silu
            with tc.tile_pool(name="inproj", bufs=1) as inproj:
                hsT_sb = inproj.tile([128, H // 128, L], F16)
                nc.sync.dma_start(hsT_sb, hsT)
                xpad = [inproj.tile([128, K - 1 + L], F16, name=f"xpad{i}")
                        for i in range(NDT)]
                xpad_evacs = [[] for _ in range(NDT)]
                for i in range(NDT):
                    nc.vector.memset(xpad[i][:, 0:K - 1], 0.0)

                for dm in range(2 * NDT):
                    for tch in range(NCH):
                        ps = psum.tile([128, LC], F32, tag="mm")
                        for k in range(H // 128):
                            nc.tensor.matmul(
                                ps,
                                lhsT=wxzT_sb[:, k, 128 * dm:128 * (dm + 1)],
                                rhs=hsT_sb[:, k, LC * tch:LC * (tch + 1)],
                                start=(k == 0), stop=(k == H // 128 - 1))
                        if dm < NDT:  # x branch -> conv input
                            ev = nc.scalar.copy(
                                xpad[dm][:, K - 1 + LC * tch:K - 1 + LC * (tch + 1)],
                                ps)
                            xpad_evacs[dm].append(ev)
                        else:         # z branch -> silu folded into the evac
                            nc.scalar.activation(
                                zsb[dm - NDT][:, LC * tch:LC * (tch + 1)], ps,
                                Act.Silu)

                # causal depthwise conv (K=4) + bias; silu on the ACT engine.
                # TensorScalarPtr-class ops only exist on DVE, so the conv
                # muls stay there (DVE is idle during the in_proj phase).
                for i in range(NDT):
                    eng = nc.vector
                    acc = inproj.tile([128, L], F16, tag="convacc", bufs=2,
                                      name="acc")
                    observe(eng, i, *xpad_evacs[i], convw_dma)
                    eng.tensor_scalar(
                        acc, xpad[i][:, 0:L], convw_sb[:, i, 0:1], None,
                        op0=Op.mult)
                    for k in range(1, K):
                        eng.scalar_tensor_tensor(
                            acc, xpad[i][:, k:k + L], convw_sb[:, i, k:k + 1],
                            acc, op0=Op.mult, op1=Op.add)
                    eng.tensor_scalar_add(acc, acc, convb_sb[:, i:i + 1])
                    nc.scalar.activation(xc[i], acc, Act.Silu)

            # scan-phase persistents allocated after inproj released its zone
            scanp = stack.enter_context(tc.tile_pool(name="scanp", bufs=1))
            brep = scanp.tile([128, L], F16, name="brep")
            crep = scanp.tile([128, L], F16, name="crep")
            ysb = [scanp.tile([128, L], F16, name=f"ysb{g}") for g in range(NDT)]
            yg = [scanp.tile([128, L], F16, name=f"yg{g}") for g in range(NDT)]

            # ---------------- phase 2: x_proj partial + AllReduce (fp16)
            # The AllReduce is split in two L-halves so dt/B/C work on half 0
            # overlaps the reduction of half 1.
            LH = L // 2
            ar_in = [dram.tile([96, LH], F16, name=f"ar_in{h}")
                     for h in range(2)]
            ar_out = [dram.tile([96, LH], F16, name=f"ar_out{h}")
                      for h in range(2)]
            for half in range(2):
                for tch in (2 * half, 2 * half + 1):
                    ps = psum.tile([128, LC], F32, tag="mm", name="ssm_ps")
                    for ki in range(NDT):
                        nc.tensor.matmul(
                            ps[0:96, :], lhsT=xpwT_sb[:, ki, :],
                            rhs=xc[ki][:, LC * tch:LC * (tch + 1)],
                            start=(ki == 0), stop=(ki == NDT - 1))
                    nc.scalar.copy(ssm16[:, LC * tch:LC * (tch + 1)],
                                   ps[0:96, :])
                nc.sync.dma_start(ar_in[half],
                                  ssm16[:, LH * half:LH * (half + 1)])
                if single_core:
                    nc.sync.dma_start(ar_out[half], ar_in[half])
                else:
                    nc.gpsimd.collective_compute(
                        "AllReduce", Op.add,
                        replica_groups=[list(range(N_CORES))],
                        ins=[ar_in[half].opt()], outs=[ar_out[half].opt()])
                nc.sync.dma_start(ssmr16[:, LH * half:LH * (half + 1)],
                                  ar_out[half])

            # ---------------- phase 3+4 per half: dt softplus, dtx, B/C rep
            for half in range(2):
                hsl = slice(LH * half, LH * (half + 1))
                for tch in (2 * half, 2 * half + 1):
                    tsl = slice(LC * tch, LC * (tch + 1))
                    for mi in range(NDT):
                        ps = psum.tile([128, LC], F32, tag="mm", name="dt_ps")
                        nc.tensor.matmul(
                            ps, lhsT=dtwT_sb[:, mi, :],
                            rhs=ssmr16[0:DTR, tsl],
                            start=True, stop=True)
                        # softplus(x+b) = ln(1 + e^(x+b)); no native softplus
                        # set in this compiler's act tables
                        spe = work.tile([128, LC], F32, tag="spe", bufs=2,
                                        name="spe")
                        nc.scalar.activation(spe, ps, Act.Exp,
                                             bias=dtb_sb[:, mi:mi + 1])
                        nc.scalar.activation(dt[mi][:, tsl], spe, Act.Ln,
                                             bias=1.0)
                    for j, dest in ((0, brep), (1, crep)):
                        ps = psum.tile([128, LC], F32, tag="mm", name="bc_ps")
                        nc.tensor.matmul(ps, lhsT=selbc_sb[:, j, :],
                                         rhs=ssmr16[:, tsl],
                                         start=True, stop=True)
                        nc.scalar.copy(dest[:, tsl], ps)
                for i in range(NDT):
                    eng = nc.vector if i == 0 else nc.gpsimd
                    eng.tensor_mul(dtx[i][:, hsl], dt[i][:, hsl],
                                   xc[i][:, hsl])

            # ---------------- phase 5: the scan
            # Full-L scans per (d,n)-tile.  Engine constraints: scans and
            # PSUM reads only exist on DVE; Pool only does plain SBUF
            # TensorTensor.  Split: scans + PSUM-reading dBx muls on DVE,
            # every hv*C mul on Pool.  The C-mul + y-reduction of tile i-2
            # are issued after tile i's expansion so the in-order PE queue
            # never stalls waiting for hc.
            def build_tile(g, li):
                i = TPG * g + li
                dA = work.tile([128, L], F16, tag="dA", bufs=3)
                last_exp = None
                for c in range(NCH):
                    tsl = slice(LC * c, LC * (c + 1))
                    drep = psum.tile([128, LC], F32, tag="mm", name="drep")
                    nc.tensor.matmul(drep, lhsT=selrep_sb[:, li, :],
                                     rhs=dt[g][:, tsl],
                                     start=True, stop=True)
                    last_exp = nc.scalar.activation(
                        dA[:, tsl], drep, Act.Exp,
                        scale=acols_sb[:, i:i + 1])
                dBx = work.tile([128, L], F16, tag="dBx", bufs=3)
                for c in range(NCH):
                    tsl = slice(LC * c, LC * (c + 1))
                    dxp = psum.tile([128, LC], F32, tag="mm", name="dxp")
                    nc.tensor.matmul(dxp, lhsT=selrep_sb[:, li, :],
                                     rhs=dtx[g][:, tsl],
                                     start=True, stop=True)
                    dbx_inst = nc.vector.tensor_mul(dBx[:, tsl], dxp,
                                                    brep[:, tsl])
                    if c == 0:
                        add_dep_helper(dbx_inst.ins, last_exp.ins, sync=True,
                                       reason="absorb ACT wait for scan")
                hv = work.tile([128, L], F16, tag="hv", bufs=3)
                nc.vector.tensor_tensor_scan(
                    hv, dA, dBx, 0.0, op0=Op.mult, op1=Op.add)
                return hv

            def reduce_tile(g, li, hv, yac):
                hc = work.tile([128, L], F16, tag="hc", bufs=3)
                nc.gpsimd.tensor_mul(hc, hv, crep)
                for c in range(NCH):
                    tsl = slice(LC * c, LC * (c + 1))
                    nc.tensor.matmul(yac[c], lhsT=selmap_sb[:, li, :],
                                     rhs=hc[:, tsl],
                                     start=(li == 0), stop=False)

            for g in range(NDT):
                if skip_scan:
                    nc.scalar.copy(ysb[g], xc[g])
                    nc.vector.tensor_mul(yg[g], ysb[g], zsb[g])
                    continue
                yac = [psum.tile([128, LC], F32, tag="yac", bufs=4,
                                 name=f"yac{c}") for c in range(NCH)]
                pending = []
                for li in range(TPG):
                    pending.append((li, build_tile(g, li)))
                    if len(pending) > 2:
                        pli, phv = pending.pop(0)
                        reduce_tile(g, pli, phv, yac)
                for pli, phv in pending:
                    reduce_tile(g, pli, phv, yac)
                for c in range(NCH):
                    tsl = slice(LC * c, LC * (c + 1))
                    # y += D * xc folded in as a diagonal matmul
                    nc.tensor.matmul(yac[c], lhsT=ddiag_sb[:, g, :],
                                     rhs=xc[g][:, tsl], start=False, stop=True)
                    nc.scalar.copy(ysb[g][:, tsl], yac[c])
                # gating for this group (overlaps the other group's scan)
                geng = nc.vector if g == 0 else nc.gpsimd
                geng.tensor_mul(yg[g], ysb[g], zsb[g])

            # ---------------- phase 6: out_proj
            for tb in range(L // 128):
                for hch in range(H // LC):
                    ops = psum.tile([128, LC], F32, tag="mm", name="out_ps")
                    for g in range(NDT):
                        nc.tensor.matmul(
                            ops, lhsT=yg[g][:, 128 * tb:128 * (tb + 1)],
                            rhs=owT_sb[:, g, LC * hch:LC * (hch + 1)],
                            start=(g == 0), stop=(g == NDT - 1))
                    osb = work.tile([128, LC], F16, tag="osb")
                    nc.scalar.copy(osb, ops)
                    nc.sync.dma_start(
                        out[128 * tb:128 * (tb + 1), LC * hch:LC * (hch + 1)],
                        osb)
    _split_multi_waits(nc)
    return nc


_NC_CACHE = None


def _get_nc():
    global _NC_CACHE
    if _NC_CACHE is None:
        _NC_CACHE = _build()
    return _NC_CACHE


# ---------------------------------------------------------------- host side
def _make_in_maps(hidden_states, in_proj_w, conv_w, conv_b, x_proj_w,
                  dt_proj_w, dt_proj_b, A_log, D, out_proj_w):
    hsT16 = np.ascontiguousarray(hidden_states[0].T, dtype=np.float16)

    # selection matrices (shared by all cores)
    p = np.arange(128)
    li = np.arange(TPG)
    k = np.arange(128)
    # SelRep[k, li, p] = 1 iff k == 8*li + p//16
    selrep = (k[:, None, None] == 8 * li[None, :, None] +
              (p // 16)[None, None, :]).astype(np.float16)
    # Selmap[p, li, m] = 1 iff m == 8*li + p//16
    selmap = (k[None, None, :] == 8 * li[None, :, None] +
              (p // 16)[:, None, None]).astype(np.float16)
    k96 = np.arange(96)
    selb = (k96[:, None] == 64 + (p % 16)[None, :])
    selc = (k96[:, None] == 80 + (p % 16)[None, :])
    selbc = np.stack([selb, selc], axis=1).astype(np.float16)

    A = -np.exp(np.asarray(A_log, np.float64))     # [DI, DS]

    in_maps = []
    for c in range(N_CORES):
        s = slice(DIL * c, DIL * (c + 1))
        wxz = np.concatenate(
            [in_proj_w[s], in_proj_w[DI + DIL * c:DI + DIL * (c + 1)]], axis=0)
        Ac = A[s]                                   # [256, 16]
        ti = np.arange(NTILE)
        acols = Ac[8 * ti[None, :] + (p // 16)[:, None], (p % 16)[:, None]]
        ddiag = np.einsum("gp,pm->pgm", D[s].reshape(NDT, 128), np.eye(128))
        # pack: order must match _build's take() calls
        parts16 = [
            hsT16,
            np.asarray(wxz.T, dtype=np.float16),
            np.asarray(out_proj_w[:, s].T, dtype=np.float16),
            np.asarray(x_proj_w[:, s].T, dtype=np.float16),
            np.asarray(dt_proj_w[s].T, dtype=np.float16),
            selrep, selmap, selbc,
            np.asarray(ddiag, np.float16),
        ]
        parts32 = [
            np.asarray(acols, np.float32),
            np.asarray(
                conv_w[s, 0, :].reshape(NDT, 128, K).transpose(1, 0, 2),
                np.float32),
            np.asarray(conv_b[s].reshape(NDT, 128).T, np.float32),
            np.asarray(dt_proj_b[s].reshape(NDT, 128).T, np.float32),
        ]
        blob16 = np.concatenate([a.ravel() for a in parts16])
        blob32 = np.concatenate([a.ravel() for a in parts32])
        assert blob16.size == BLOB16_SIZE and blob32.size == BLOB32_SIZE
        in_maps.append({"blob16": blob16, "blob32": blob32})
    return in_maps


def kernel(hidden_states, in_proj_w, conv_w, conv_b, x_proj_w,
           dt_proj_w, dt_proj_b, A_log, D, out_proj_w):
    args = [np.asarray(a, np.float32) for a in
            (hidden_states, in_proj_w, conv_w, conv_b, x_proj_w,
             dt_proj_w, dt_proj_b, A_log, D, out_proj_w)]
    in_maps = _make_in_maps(*args)
    nc = _get_nc()
    res = run_bass_kernel_spmd(nc, in_maps, core_ids=list(range(N_CORES)))
    out = np.zeros((L, H), np.float64)
    for r in res.results:
        out += r["out"].astype(np.float64)
    return out.astype(np.float32).reshape(B, L, H)
